# revision 41
# baseline (speedup 1.0000x reference)
"""GCN (3-layer message passing) distributed over 8 TRN2 NeuronCores.

Sharding: nodes split evenly across 8 cores (rows). Weights replicated.
Per layer: local matmul h = x @ W + b (node-major out via x^T-stationary
matmuls), gather of the rows each peer needs into per-position-chunk
AllToAll send buffers; chunk k's AllToAll fires as soon as its 8 sends
are staged (overlapping the exchange with the matmul still producing
rows for later chunks); then a local segment-sum implemented as one-hot
matmuls (edges tiled 128 at a time, PSUM-accumulated per 128-node
destination window), with relu fused in the epilogue. One-hot tiles are
precomputed host-side and kept SBUF-resident (shared by all layers).
A warm-up barrier collective at kernel start absorbs cross-core launch
skew under mm0 so the first data exchange pays no rendezvous.

Measured on trn2: an 8-core AllToAll moves data at a fixed ~104 GB/s
protocol rate with ~10-80us fixed latency per collective and ~80-100us
turnaround between back-to-back collectives, so few, early-fired
collectives beat fine-grained chunking.

Layer 3's inclusion linear Wi is folded into W3 (segment_sum commutes
with right-matmul), so the last exchange is only 16 (padded 128) wide.

Everything data-dependent (pair row counts, edge tiling) is computed
host-side in preprocess(); all 8 cores run one SPMD graph whose shapes
depend only on those computed constants.
"""
import sys

sys.path.insert(0, "/opt/trn_rl_repo")

import numpy as np
import ml_dtypes

import concourse.bass as bass
import concourse.bacc as bacc
import concourse.mybir as mybir
import concourse.tile as tile
from concourse.bass_utils import run_bass_kernel_spmd

NC = 8
BF16 = mybir.dt.bfloat16
F32 = mybir.dt.float32
I16 = mybir.dt.int16

# column chunking of each layer's exchange/aggregation (chunking the
# AllToAll measured strictly worse: each collective costs a fixed
# ~72-150us regardless of size, so one exchange per layer is optimal)
CHUNKS = [[0, 768], [0, 512], [0, 128]]

last_exec_time_ns = None
last_results = None


def _wrap16(idx, ncols):
    """[n] int -> [128, n/16] int16 wrapped (idx i at [i%16, i//16]) and
    replicated to 128 partitions."""
    a = np.asarray(idx, np.int16).reshape(ncols, 16).T  # [16, n/16]
    return np.tile(a, (8, 1))


def preprocess(features, W1, b1, W2, b2, W3, b3, Wi, bi, src, dst):
    """Host-side sharding/setup. Returns (cfg dict, in_maps list).

    Pad rows of each aggregation window get idx -1 and are skipped via
    num_idxs_reg; their one-hot columns are all-zero so stale gather
    lanes contribute nothing."""
    N, K1t = features.shape  # 50000, 1433
    E = src.shape[0]
    assert N % NC == 0
    NLOC = N // NC
    MBLK = (NLOC + 127) // 128
    NPAD = MBLK * 128

    TW = [768, 512, 128]          # h-table widths (bf16, 256B-aligned)
    K = [1536, TW[0], TW[1]]      # matmul contraction dims (128-aligned)
    KB = [k // 128 for k in K]

    # ---- weights (fold Wi into W3), padded, bf16 ----
    W3f = (W3.astype(np.float64) @ Wi.astype(np.float64)).astype(np.float32)
    b3f = (b3.astype(np.float64) @ Wi.astype(np.float64)).astype(np.float32)

    def pad2(a, r, c):
        out = np.zeros((r, c), np.float32)
        out[: a.shape[0], : a.shape[1]] = a
        return out

    w1 = pad2(W1, K[0], TW[0]).astype(ml_dtypes.bfloat16)
    w2 = pad2(W2, K[1], TW[1]).astype(ml_dtypes.bfloat16)
    w3 = pad2(W3f, K[2], TW[2]).astype(ml_dtypes.bfloat16)
    b1p = np.tile(pad2(b1[None, :], 1, TW[0]), (128, 1))
    b2p = np.tile(pad2(b2[None, :], 1, TW[1]), (128, 1))
    b3p = np.tile(pad2(b3f[None, :], 1, TW[2]), (128, 1))
    # bi replicated across the MBLK output windows for one batched epilogue
    bip = np.tile(np.tile(bi[None, :16], (1, MBLK)), (128, 1)).astype(np.float32)

    # ---- per-core transposed features [K[0], NPAD] bf16 ----
    featTs = []
    for c in range(NC):
        ft = np.zeros((K[0], NPAD), np.float32)
        ft[:K1t, :NLOC] = features[c * NLOC : (c + 1) * NLOC].T
        featTs.append(ft.astype(ml_dtypes.bfloat16))

    # ---- graph structure ----
    src = np.asarray(src, np.int64)
    dst = np.asarray(dst, np.int64)
    owner = src // NLOC
    dcore = dst // NLOC

    # unique sources per (owner o -> dest d) pair
    uniq = [[None] * NC for _ in range(NC)]
    for d in range(NC):
        maskd = dcore == d
        for o in range(NC):
            m = maskd & (owner == o)
            uniq[o][d] = np.unique(src[m])
    P = max(len(uniq[o][d]) for o in range(NC) for d in range(NC))
    P = ((P + 127) // 128) * 128

    # Send-gather call plan: each dest pair's sorted row list is split into
    # calls of <=1024 positions. Rows within a call span a narrow range, so
    # the gather's table AP can be sliced to static [r0, r1) bounds (min/max
    # over cores) -- Tile's range deps then let early calls start while the
    # matmul is still writing later h rows. Pads (idx -1) are skipped via a
    # runtime count register.
    # The exchange is chunked by position range k: chunk k's AllToAll fires
    # as soon as all dests' chunk-k sends are staged (while the matmul is
    # still producing rows needed by later chunks), hiding the transfer.
    # recv layout is chunk-major: row(o, pos) = NC*base_k + o*csz_k +
    # (pos - base_k).
    CSZ = 1024
    csizes = []
    off = 0
    while off < P:
        csizes.append(min(CSZ, P - off))
        off += CSZ
    cbase = np.concatenate([[0], np.cumsum(csizes)]).astype(np.int64)
    # row bounds per (dest, call): min/max across cores
    NSC = NC * len(csizes)
    r0s = np.full(NSC, NPAD, np.int64)
    r1s = np.zeros(NSC, np.int64)
    for o in range(NC):
        for d in range(NC):
            u = uniq[o][d] - o * NLOC
            off = 0
            for k, csz in enumerate(csizes):
                seg = u[off : off + csz]
                ci = d * len(csizes) + k
                if len(seg):
                    r0s[ci] = min(r0s[ci], seg[0])
                    r1s[ci] = max(r1s[ci], seg[-1] + 1)
                off += csz
    r0s = np.minimum(r0s, r1s)  # empty calls -> [0, 0) -> fix below
    r1s = np.maximum(r1s, r0s + 1)
    scalls = []  # (ci, k, d, csz, r0, r1)
    for d in range(NC):
        for k, csz in enumerate(csizes):
            ci = d * len(csizes) + k
            scalls.append((ci, k, d, csz, int(r0s[ci]), int(r1s[ci])))

    # send gather index stream per core o (rebased by call r0, pads = -1)
    sidx, scnts = [], []
    for o in range(NC):
        parts, cnts = [], []
        for d in range(NC):
            u = uniq[o][d] - o * NLOC
            off = 0
            for k, csz in enumerate(csizes):
                ci = d * len(csizes) + k
                seg = u[off : off + csz] - r0s[ci]
                parts.append(np.concatenate(
                    [seg, np.full(csz - len(seg), -1, np.int64)]))
                cnts.append(len(seg))
                off += csz
        sidx.append(_wrap16(np.concatenate(parts), NC * P // 16))
        scnts.append(np.asarray(cnts, np.int32).reshape(1, NSC))

    # edge tiling per dest core: tiles of 128 edges within 128-dst windows
    per_core = []
    for d in range(NC):
        m = dcore == d
        es, ed = src[m], dst[m]
        eo = es // NLOC
        pos = np.empty(len(es), np.int64)
        for o in range(NC):
            mo = eo == o
            pos[mo] = np.searchsorted(uniq[o][d], es[mo])
        # chunk-major recv layout
        kk = np.minimum(pos // CSZ, len(csizes) - 1)
        csz_k = np.asarray(csizes, np.int64)[kk]
        tbl = NC * cbase[kk] + eo * csz_k + (pos - cbase[kk])
        ldst = ed - d * NLOC
        win = ldst // 128
        rel = ldst % 128
        per_core.append((tbl, win, rel))

    # common tiles-per-window across cores
    NW = MBLK
    Tw = np.ones(NW, np.int64)
    for d in range(NC):
        _, win, _ = per_core[d]
        cnt = np.bincount(win, minlength=NW)
        Tw = np.maximum(Tw, (cnt + 127) // 128)
    tile_start = np.concatenate([[0], np.cumsum(Tw)])
    T8 = ((int(tile_start[-1]) + 7) // 8) * 8

    gidxs, ohs, acnts = [], [], []
    for d in range(NC):
        tbl, win, rel = per_core[d]
        order = np.lexsort((tbl, win))
        tbl, win, rel = tbl[order], win[order], rel[order]
        g = np.full(T8 * 128, -1, np.int64)
        r = np.full(T8 * 128, -1, np.int64)
        cnts = np.zeros(NW, np.int32)
        for w in range(NW):
            m = win == w
            n = int(m.sum())
            cnts[w] = n
            off = int(tile_start[w]) * 128
            g[off : off + n] = tbl[m]
            r[off : off + n] = rel[m]
        gidxs.append(_wrap16(g, T8 * 8))
        # one-hot table [128 edge-lane, T8 tiles, 128 dst-rel] bf16
        oh = np.zeros((T8 * 128, 128), np.float32)
        valid = r >= 0
        oh[np.nonzero(valid)[0], r[valid]] = 1.0
        ohs.append(np.ascontiguousarray(
            oh.reshape(T8, 128, 128).transpose(1, 0, 2)
        ).astype(ml_dtypes.bfloat16).reshape(128, T8 * 128))
        acnts.append(cnts.reshape(1, NW))

    cfg = dict(NLOC=NLOC, NPAD=NPAD, MBLK=MBLK, TW=TW, K=K, KB=KB, P=P,
               scalls=scalls, NSC=NSC, csizes=[int(x) for x in csizes],
               cbase=[int(x) for x in cbase],
               Tw=[int(x) for x in Tw],
               tile_start=[int(x) for x in tile_start],
               T8=T8, OUT_W=16)

    in_maps = []
    for c in range(NC):
        in_maps.append({
            "featT": featTs[c],
            "w1": w1, "w2": w2, "w3": w3,
            "b1": b1p, "b2": b2p, "b3": b3p, "bi": bip,
            "sidx": sidx[c], "gidx": gidxs[c], "ohtab": ohs[c],
            "acnt": acnts[c], "scnt": scnts[c],
        })
    return cfg, in_maps


def build(cfg, nq=4):
    NLOC, NPAD, MBLK = cfg["NLOC"], cfg["NPAD"], cfg["MBLK"]
    TW, K, KB, P = cfg["TW"], cfg["K"], cfg["KB"], cfg["P"]
    Tw, tile_start, T8 = cfg["Tw"], cfg["tile_start"], cfg["T8"]
    scalls, NSC = cfg["scalls"], cfg["NSC"]
    OUT_W = cfg["OUT_W"]
    NW = MBLK

    AGP_BUFS = 4
    nc = bacc.Bacc("TRN2", target_bir_lowering=False, debug=False,
                   num_devices=NC, num_swdge_queues=nq)

    featT = nc.declare_dram_parameter("featT", [K[0], NPAD], BF16, isOutput=False)
    wts = [nc.declare_dram_parameter(f"w{l+1}", [K[l], TW[l]], BF16, isOutput=False)
           for l in range(3)]
    bs = [nc.declare_dram_parameter(f"b{l+1}", [128, TW[l]], F32, isOutput=False)
          for l in range(3)]
    bi = nc.declare_dram_parameter("bi", [128, MBLK * OUT_W], F32, isOutput=False)
    sidx = nc.declare_dram_parameter("sidx", [128, NC * P // 16], I16, isOutput=False)
    gidx = nc.declare_dram_parameter("gidx", [128, T8 * 8], I16, isOutput=False)
    ohtab = nc.declare_dram_parameter("ohtab", [128, T8 * 128], BF16, isOutput=False)
    acnt = nc.declare_dram_parameter("acnt", [1, MBLK], mybir.dt.int32, isOutput=False)
    scnt = nc.declare_dram_parameter("scnt", [1, NSC], mybir.dt.int32, isOutput=False)
    out = nc.declare_dram_parameter("out", [NLOC, OUT_W], F32, isOutput=True)

    csizes, cbase = cfg["csizes"], cfg["cbase"]
    hloc = [nc.dram_tensor(f"hloc{l}", [NPAD, TW[l]], BF16) for l in range(3)]
    # position-chunked exchange: one send buffer per chunk, one recv table
    # per layer (chunk-major row layout)
    a2ain = [[nc.dram_tensor(f"a2ain{l}_{k}", [NC * csz, TW[l]], BF16)
              for k, csz in enumerate(csizes)] for l in range(3)]
    recv = [nc.dram_tensor(f"recv{l}", [NC * P, TW[l]], BF16) for l in range(3)]
    xs = [None, nc.dram_tensor("x2", [NPAD, TW[0]], BF16),
          nc.dram_tensor("x3", [NPAD, TW[1]], BF16)]
    warm_in = nc.dram_tensor("warm_in", [NC * 128, 16], BF16)
    warm_out = nc.dram_tensor("warm_out", [NC * 128, 16], BF16)

    groups = [list(range(NC))]

    with tile.TileContext(nc) as tc:
        with (
            tc.tile_pool(name="wpool", bufs=1) as wpool,
            tc.tile_pool(name="bpool", bufs=1) as bpool,
            tc.tile_pool(name="ipool", bufs=1) as ipool,
            tc.tile_pool(name="xtp", bufs=2) as xtp,
            tc.tile_pool(name="mmpsum", bufs=2, space="PSUM") as mmpsum,
            tc.tile_pool(name="hbp", bufs=3) as hbp,
            tc.tile_pool(name="sgp", bufs=4) as sgp,
            tc.tile_pool(name="agp", bufs=AGP_BUFS) as agp,
            tc.tile_pool(name="apsum", bufs=2, space="PSUM") as apsum,
            tc.tile_pool(name="xop", bufs=3) as xop,
        ):
            # resident: indices, one-hot table
            sidx_t = ipool.tile([128, NC * P // 16], I16, tag="sidx")
            nc.sync.dma_start(sidx_t[:], sidx[:])
            gidx_t = ipool.tile([128, T8 * 8], I16, tag="gidx")
            nc.sync.dma_start(gidx_t[:], gidx[:])
            oh_t = ipool.tile([128, T8, 128], BF16, tag="ohtab")
            nc.sync.dma_start(oh_t[:], ohtab.rearrange("p (t d) -> p t d", d=128))
            obuf = ipool.tile([128, NW, OUT_W], F32, tag="obuf")
            acnt_t = ipool.tile([1, NW], mybir.dt.int32, tag="acnt")
            nc.sync.dma_start(acnt_t[:], acnt[:])
            scnt_t = ipool.tile([1, NSC], mybir.dt.int32, tag="scnt")
            nc.sync.dma_start(scnt_t[:], scnt[:])
            TWMAX = max(Tw)
            CWMAX = max(CHUNKS[0][c + 1] - CHUNKS[0][c] for c in range(len(CHUNKS[0]) - 1))
            # warm-up barrier: absorbs cross-core launch skew while mm0 runs,
            # so L0's data exchange doesn't pay the rendezvous
            nc.gpsimd.collective_compute(
                "AllToAll", mybir.AluOpType.bypass, replica_groups=groups,
                ins=[warm_in[:]], outs=[warm_out[:]])
            # zero the gather slots once so rows skipped by short gathers
            # (num_idxs_reg < num_idxs) read as finite values
            for _ in range(AGP_BUFS):
                zt = agp.tile([128, TWMAX, CWMAX], BF16, tag="ag")
                nc.vector.memset(zt[:], 0.0)
            nreg = nc.gpsimd.alloc_register()

            for l in range(3):
              nch = len(CHUNKS[l]) - 1
              with nc.named_scope(f"L{l}"):
                  # ---- resident weights/bias for this layer ----
                  wt = wpool.tile([128, KB[l], TW[l]], BF16, tag="w")
                  nc.sync.dma_start(
                      wt[:], wts[l].rearrange("(kb p) w -> p kb w", p=128))
                  bt = bpool.tile([128, TW[l]], F32, tag="b")
                  nc.sync.dma_start(bt[:], bs[l][:])
                  if l == 2:
                      bit = bpool.tile([128, NW * OUT_W], F32, tag="bi")
                      nc.sync.dma_start(bit[:], bi[:])

                  # ---- matmul: h = x @ W + b  (node-major PSUM out) ----
                  nslices = [(s, min(s + 512, TW[l])) for s in range(0, TW[l], 512)]
                  sc_mm = nc.enter_named_scope(f"mm{l}", False)[0]
                  NRW = 512
                  for nr in range(0, NPAD, NRW):
                      rw = min(NRW, NPAD - nr)
                      stripes = []
                      for kb in range(KB[l]):
                          st = xtp.tile([128, NRW], BF16, tag=f"xt{kb}")
                          if l == 0:
                              nc.sync.dma_start(
                                  st[:, :rw],
                                  featT[kb * 128 : (kb + 1) * 128, nr : nr + rw])
                          else:
                              nc.sync.dma_start_transpose(
                                  st[:, :rw],
                                  xs[l][nr : nr + rw, kb * 128 : (kb + 1) * 128])
                          stripes.append(st)
                      for m in range(rw // 128):
                          ps = mmpsum.tile([128, TW[l]], F32, tag="mmps")
                          for kb in range(KB[l]):
                              for (s0, s1) in nslices:
                                  nc.tensor.matmul(
                                      ps[:, s0:s1],
                                      stripes[kb][:, m * 128 : (m + 1) * 128],
                                      wt[:, kb, s0:s1],
                                      start=(kb == 0), stop=(kb == KB[l] - 1))
                          hb = hbp.tile([128, TW[l]], BF16, tag="hb")
                          nc.vector.tensor_tensor(
                              hb[:], ps[:], bt[:], op=mybir.AluOpType.add)
                          nc.sync.dma_start(
                              hloc[l][nr + m * 128 : nr + (m + 1) * 128, :], hb[:])

                  nc.leave_named_scope(f"mm{l}", sc_mm, False)
                  # ---- send gather + position-chunked exchange ----
                  sc_sg = nc.enter_named_scope(f"sg{l}", False)[0]
                  # k-major order: calls needing only early h rows first; as
                  # soon as chunk k's 8 sends are staged, its AllToAll fires
                  # (overlapping the matmul still producing later h rows)
                  for (ci, k, d, csz, r0, r1) in sorted(
                          scalls, key=lambda t: (t[1], t[2])):
                      # sidx stream position for this call
                      pos0 = d * P + cbase[k]
                      g = sgp.tile([128, max(csizes) // 128, TW[l]], BF16,
                                   tag="sg")
                      nb = csz // 128
                      nc.gpsimd.reg_load(nreg, scnt_t[0:1, ci : ci + 1])
                      nc.gpsimd.dma_gather(
                          g[:, :nb, :], hloc[l][r0:r1],
                          sidx_t[:, pos0 // 16 : (pos0 + csz) // 16],
                          csz, nreg, TW[l], queue_num=ci % nq)
                      nc.sync.dma_start(
                          a2ain[l][k][d * csz : (d + 1) * csz, :]
                          .rearrange("(b p) w -> p b w", p=128),
                          g[:, :nb, :])
                      if d == NC - 1:
                          o0 = NC * cbase[k]
                          nc.gpsimd.collective_compute(
                              "AllToAll", mybir.AluOpType.bypass,
                              replica_groups=groups,
                              ins=[a2ain[l][k][:]],
                              outs=[recv[l][o0 : o0 + NC * csz, :]])
                  nc.leave_named_scope(f"sg{l}", sc_sg, False)
                  # ---- aggregation: segment-sum via one-hot matmuls ----
                  # chunk-pass-major: pass c runs while chunk c+1's AllToAll
                  # is still in flight
                  sc_ag = nc.enter_named_scope(f"agg{l}", False)[0]
                  for c in range(nch):
                      c0, c1 = CHUNKS[l][c], CHUNKS[l][c + 1]
                      cw = c1 - c0
                      for w in range(NW):
                          ps = apsum.tile([128, cw], F32, tag="aps")
                          t0 = tile_start[w]
                          gt = agp.tile([128, TWMAX, cw], BF16, tag="ag")
                          nc.gpsimd.reg_load(nreg, acnt_t[0:1, w : w + 1])
                          nc.gpsimd.dma_gather(
                              gt[:, : Tw[w], :], recv[l][:],
                              gidx_t[:, t0 * 8 : (t0 + Tw[w]) * 8],
                              Tw[w] * 128, nreg, cw, queue_num=w % nq)
                          cslices = [(s, min(s + 512, cw))
                                     for s in range(0, cw, 512)]
                          for tl in range(Tw[w]):
                              t = t0 + tl
                              for (s0, s1) in cslices:
                                  nc.tensor.matmul(
                                      ps[:, s0:s1], oh_t[:, t, :],
                                      gt[:, tl, s0:s1],
                                      start=(tl == 0), stop=(tl == Tw[w] - 1))
                          # ---- epilogue ----
                          if l < 2:
                              xb = xop.tile([128, cw], BF16, tag="xo")
                              nc.vector.tensor_scalar_max(xb[:], ps[:], 0.0)
                              nc.sync.dma_start(
                                  xs[l + 1][w * 128 : (w + 1) * 128, c0:c1],
                                  xb[:])
                          else:
                              nc.vector.tensor_scalar(
                                  obuf[:, w, :], ps[:, :OUT_W], 0.0, None,
                                  mybir.AluOpType.add)
                  if l == 2:
                      # batched bias + relu over all windows at once
                      nc.vector.tensor_tensor(
                          obuf[:], obuf[:], bit[:].rearrange(
                              "p (w c) -> p w c", c=OUT_W),
                          op=mybir.AluOpType.add)
                      nc.vector.tensor_scalar_max(obuf[:], obuf[:], 0.0)
                      # one batched store for the full windows, then the tail
                      WFULL = NLOC // 128
                      nc.sync.dma_start(
                          out[: WFULL * 128, :]
                          .rearrange("(w p) c -> p w c", p=128),
                          obuf[:, :WFULL, :])
                      rows = NLOC - WFULL * 128
                      if rows > 0:
                          nc.sync.dma_start(
                              out[WFULL * 128 :, :], obuf[:rows, WFULL, :])
                  nc.leave_named_scope(f"agg{l}", sc_ag, False)
    nc.finalize()
    return nc


def kernel(**inputs):
    global last_exec_time_ns, last_results
    inputs = {k: np.asarray(v) for k, v in inputs.items()}
    cfg, in_maps = preprocess(**inputs)
    nc = build(cfg)
    res = None
    # trace=True needs the axon NTFF hook; fall back to untraced runs, and
    # retry once more on transient device errors (NRT_EXEC_UNIT_UNRECOVERABLE).
    for attempt, trace in enumerate([True, False, False]):
        try:
            res = run_bass_kernel_spmd(
                nc, in_maps, core_ids=list(range(NC)), trace=trace)
            break
        except Exception:
            if attempt == 2:
                raise
            import time
            time.sleep(15)
    last_exec_time_ns = res.exec_time_ns
    last_results = res
    return np.concatenate([res.results[c]["out"] for c in range(NC)], axis=0)
